# revision 26
# baseline (speedup 1.0000x reference)
"""DeepseekV2 MLA attention fusion (prefill, T=2048) on 8 Trainium2 cores.

Sharding:
  - Phase A (q_a / kv_a projections + rmsnorm + k_pe rope): sharded over tokens
    (256 tokens/core), feature-major outputs.  kv/k_pe AllGathered early
    (bf16), q_c AllGathered in two row-halves as fp8e4 (the rmsnormed q_c is
    O(1), e4m3 transport keeps rel err ~1e-2) so the dominant collective is
    half the bytes.
  - Phase B tensor-parallel over heads (2 heads/core).  kv_b expansion (B2)
    overlaps the q AllGather; q_b projection (B1) consumes the fp8 q_c
    directly as the matmul moving operand, with a packed rope stationary
    (3 matmuls/k-tile for 384 output cols) and natural k-split accumulation
    so its first half starts as soon as agq half-a lands.
  - Attention runs per 512-token j-supertile in DESCENDING order (j3..j0);
    each j's normalized output is AllGathered (bf16) immediately and its
    o_proj (feature-sharded, 256 cols/core) queues behind the remaining
    attention work, so the final gather overlaps queued o_proj matmuls.
  - softmax denominator: fp32 vector accumulation of the exp tiles + a single
    ones-matmul per (j,h) instead of a PE matmul per s-tile.

All matmuls bf16 (q_b: bf16 stationary x fp8 moving) with fp32 PSUM
accumulation; softmax in fp32.
"""
import sys

sys.path.insert(0, "/opt/trn_rl_repo")

from contextlib import ExitStack

import ml_dtypes
import numpy as np

from concourse import bacc, bass, mybir, tile
from concourse.bass_utils import run_bass_kernel_spmd

BF16 = mybir.dt.bfloat16
F32 = mybir.dt.float32
FP8 = mybir.dt.float8e4
NPBF16 = ml_dtypes.bfloat16
NPFP8 = ml_dtypes.float8_e4m3

T = 2048
HID = 2048
H = 16
DN = 128
DR = 64
DV = 128
QL = 1536
KVL = 512
KVR = KVL + DR
THETA = 10000.0
EPS = 1e-6
SCALE = (DN + DR) ** -0.5

NC = 8            # cores
TC = T // NC      # tokens per core = 256
HC = H // NC      # heads per core = 2
EC = HID // NC    # o_proj output cols per core = 256


def build_program():
    nc = bacc.Bacc("TRN2", target_bir_lowering=False, debug=False, num_devices=NC)

    _ctr = [0]

    def nm(tag):
        _ctr[0] += 1
        return f"t{_ctr[0]}_{tag}"

    def din(name, shape, dt=BF16):
        return nc.dram_tensor(name, shape, dt, kind="ExternalInput").ap()

    hid_T = din("hid_T", [HID, TC])                 # hidden.T chunk, p-major rows
    w_q_aT = din("w_q_aT", [48 * 128, 512])         # packed (J,k) tiles
    w_kv_aT = din("w_kv_aT", [HID, KVR])            # rope cols de-interleaved
    w_qbT = din("w_qbT", [QL, 3 * 128])             # [h0n, h1n, h0p|h1p] cols
    w_kbT = din("w_kbT", [KVL, 2 * DN])             # [h0 k, h1 k]
    w_vbT = din("w_vbT", [KVL, 2 * DV])             # [h0 v, h1 v]
    w_oT = din("w_oT", [H * DV, EC])
    cos_t = din("cos_t", [DR // 2, T], F32)         # [32, 2048] (q rope)
    sin_t = din("sin_t", [DR // 2, T], F32)
    cos_c = din("cos_c", [DR // 2, TC], F32)        # core's chunk (k rope)
    sin_c = din("sin_c", [DR // 2, TC], F32)
    masks = din("masks", [16, 128, 128])            # diag triangles

    y_out = nc.dram_tensor("y_out", [EC, T], F32, kind="ExternalOutput").ap()

    agkv_in = nc.dram_tensor("agkv_in", [KVR, TC], BF16).ap()
    agkv_out = nc.dram_tensor("agkv_out", [NC * KVR, TC], BF16,
                              addr_space="Shared").ap()
    agq_in = nc.dram_tensor("agq_in", [QL, TC], FP8).ap()
    agq_out = nc.dram_tensor("agq_out", [NC * QL, TC], FP8,
                             addr_space="Shared").ap()
    ag2_in = nc.dram_tensor("ag2a_in", [HC * DV, 1024], BF16).ap()
    ag2_out = nc.dram_tensor("ag2a_out", [H * DV, 1024], BF16,
                             addr_space="Shared").ap()
    ag2j_in = {j: nc.dram_tensor(f"ag2{j}_in", [HC * DV, 512], BF16).ap()
               for j in (2, 3)}
    ag2j_out = {j: nc.dram_tensor(f"ag2{j}_out", [H * DV, 512], BF16,
                                  addr_space="Shared").ap() for j in (2, 3)}

    rg = [list(range(NC))]

    with tile.TileContext(nc) as tc, ExitStack() as ctx:
        const = ctx.enter_context(tc.tile_pool(name="const", bufs=1))
        sp = ctx.enter_context(tc.tile_pool(name="sp", bufs=2))

        ones_bf = const.tile([128, 1], BF16, tag="ones_bf", name=nm("ones_bf"))
        nc.vector.memset(ones_bf[:], 1.0)

        # ============ Phase A (token-sharded, feature-major outputs) =======
        ones_col = const.tile([128, 1], F32, tag="ones_col", name=nm("ones_col"))
        nc.vector.memset(ones_col[:], 1.0)
        eps1_sb = const.tile([1, 1], F32, tag="eps1", name=nm("eps1"))
        nc.vector.memset(eps1_sb[:], EPS)

        with tc.tile_pool(name="pA", bufs=1) as pA, \
             tc.tile_pool(name="psA", bufs=1, space="PSUM") as psA:
            # rows pre-permuted host-side: dram row (16 p + k) = contraction
            # row (128 k + p), so each chunk is contiguous per partition.
            hid4 = hid_T.rearrange("(p g k) t -> p g (k t)", p=128, g=4)
            hid_sb = []
            for g in range(4):
                t = pA.tile([128, 4 * TC], BF16, tag=f"hid{g}",
                            name=nm(f"hid{g}"))
                nc.sync.dma_start(out=t[:], in_=hid4[:, g, :])
                hid_sb.append(t)

            def hid_k(k):
                g, kk = divmod(k, 4)
                return hid_sb[g][:, TC * kk:TC * (kk + 1)]

            def norm_scale(sumsq, n_feat):
                """rsqrt(mean+eps) broadcast to [128, TC] SBUF tile."""
                s = sp.tile([1, TC], F32, tag="nrm", bufs=2, name=nm("nrm"))
                nc.scalar.activation(s[:], sumsq[:],
                                     mybir.ActivationFunctionType.Sqrt,
                                     bias=eps1_sb[:], scale=1.0 / n_feat)
                r = sp.tile([1, TC], F32, tag="nrm_r", bufs=2, name=nm("nrm_r"))
                nc.vector.reciprocal_approx_fast(r[:], s[:])
                bc = sp.tile([128, TC], F32, tag="nrm_bc", bufs=2,
                             name=nm("nrm_bc"))
                nc.gpsimd.partition_broadcast(bc[:], r[:])
                return bc

            # ---- latent: kv_c 4 blocks + k_pe, fused per-k loop ----
            kv_ps = [psA.tile([128, TC], F32, tag="a_ps", bufs=4,
                              name=nm("a_ps")) for _ in range(4)]
            pe_ps = psA.tile([64, TC], F32, tag="pe_ps", bufs=1, name=nm("pe_ps"))
            for k in range(16):
                wt = sp.tile([128, KVR], BF16, tag="wA", bufs=4, name=nm("wA"))
                nc.scalar.dma_start(out=wt[:],
                                    in_=w_kv_aT[128 * k:128 * (k + 1), :])
                for jj in range(4):
                    nc.tensor.matmul(kv_ps[jj][:], wt[:, 128 * jj:128 * (jj + 1)],
                                     hid_k(k), start=(k == 0), stop=(k == 15))
                nc.tensor.matmul(pe_ps[:], wt[:, KVL:KVR], hid_k(k),
                                 start=(k == 0), stop=(k == 15))
            kv_sq = psA.tile([1, TC], F32, tag="sumsq", bufs=2, name=nm("sumsq"))
            kvc_sb = []
            for jj in range(4):
                sq = sp.tile([128, TC], F32, tag="sq", bufs=2, name=nm("sq"))
                nc.scalar.square(sq[:], kv_ps[jj][:])
                o = pA.tile([128, TC], F32, tag=f"kvc{jj}", name=nm("kvc"))
                nc.vector.tensor_copy(o[:], kv_ps[jj][:])
                kvc_sb.append(o)
                nc.tensor.matmul(kv_sq[:], ones_col[:], sq[:],
                                 start=(jj == 0), stop=(jj == 3))
            kbc = norm_scale(kv_sq, KVL)
            for jj in range(4):
                o = sp.tile([128, TC], BF16, tag="agst", bufs=3, name=nm("agst"))
                nc.vector.tensor_mul(o[:], kvc_sb[jj][:], kbc[:])
                nc.sync.dma_start(out=agkv_in[128 * jj:128 * (jj + 1), :],
                                  in_=o[:])
            # k_pe rope (partition pairs 0:32 / 32:64, de-interleaved)
            cosc = pA.tile([32, TC], F32, tag="cosc", name=nm("cosc"))
            sinc = pA.tile([32, TC], F32, tag="sinc", name=nm("sinc"))
            nc.sync.dma_start(out=cosc[:], in_=cos_c[:])
            nc.sync.dma_start(out=sinc[:], in_=sin_c[:])
            kpe_ro = sp.tile([64, TC], BF16, tag="kpero", bufs=1, name=nm("kpero"))
            t1 = sp.tile([32, TC], F32, tag="ro1", bufs=2, name=nm("ro1"))
            t2 = sp.tile([32, TC], F32, tag="ro2", bufs=2, name=nm("ro2"))
            nc.vector.tensor_mul(t1[:], pe_ps[0:32, :], cosc[:])
            nc.vector.tensor_mul(t2[:], pe_ps[32:64, :], sinc[:])
            nc.vector.tensor_sub(kpe_ro[0:32, :], t1[:], t2[:])
            t3 = sp.tile([32, TC], F32, tag="ro1", bufs=2, name=nm("ro1"))
            t4 = sp.tile([32, TC], F32, tag="ro2", bufs=2, name=nm("ro2"))
            nc.vector.tensor_mul(t3[:], pe_ps[32:64, :], cosc[:])
            nc.vector.tensor_mul(t4[:], pe_ps[0:32, :], sinc[:])
            nc.vector.tensor_add(kpe_ro[32:64, :], t3[:], t4[:])
            nc.sync.dma_start(out=agkv_in[KVL:KVR, :], in_=kpe_ro[:])

            nc.gpsimd.collective_compute(
                "AllGather", mybir.AluOpType.bypass, replica_groups=rg,
                ins=[agkv_in[:]], outs=[agkv_out[:]])

            # ---- q_c: 12 blocks in 3 J-supertiles; fp8 transport ----
            q_sq = psA.tile([1, TC], F32, tag="sumsq", bufs=2, name=nm("sumsq"))
            qc_sb = []
            for J in range(3):
                pss = [psA.tile([128, TC], F32, tag="a_ps", bufs=4,
                                name=nm("a_ps")) for _ in range(4)]
                for k in range(16):
                    wt = sp.tile([128, 512], BF16, tag="wAq", bufs=4,
                                 name=nm("wAq"))
                    r0 = (16 * J + k) * 128
                    eng = nc.sync if k % 2 == 0 else nc.gpsimd
                    eng.dma_start(out=wt[:], in_=w_q_aT[r0:r0 + 128, :])
                    for jj in range(4):
                        nc.tensor.matmul(pss[jj][:],
                                         wt[:, 128 * jj:128 * (jj + 1)],
                                         hid_k(k), start=(k == 0), stop=(k == 15))
                for jj in range(4):
                    j = 4 * J + jj
                    sq = sp.tile([128, TC], F32, tag="sq", bufs=2, name=nm("sq"))
                    nc.scalar.square(sq[:], pss[jj][:])
                    o = pA.tile([128, TC], F32, tag=f"qc{j}", name=nm("qc"))
                    nc.vector.tensor_copy(o[:], pss[jj][:])
                    qc_sb.append(o)
                    nc.tensor.matmul(q_sq[:], ones_col[:], sq[:],
                                     start=(j == 0), stop=(j == 11))
            qbc = norm_scale(q_sq, QL)
            for j in range(12):
                o = sp.tile([128, TC], FP8, tag="agstq", bufs=3,
                            name=nm("agstq"))
                nc.vector.tensor_mul(o[:], qc_sb[j][:], qbc[:])
                nc.sync.dma_start(
                    out=agq_in[128 * j:128 * (j + 1), :], in_=o[:])
            nc.gpsimd.collective_compute(
                "AllGather", mybir.AluOpType.bypass, replica_groups=rg,
                ins=[agq_in[:]], outs=[agq_out[:]])

        # ============ Phase B resident tensors =============================
        pB = ctx.enter_context(tc.tile_pool(name="pB", bufs=1))
        cos_sb = pB.tile([32, T], F32, tag="cos", name=nm("cos"))
        sin_sb = pB.tile([32, T], F32, tag="sin", name=nm("sin"))
        nc.gpsimd.dma_start(out=cos_sb[:], in_=cos_t[:])
        nc.gpsimd.dma_start(out=sin_sb[:], in_=sin_t[:])

        wkb_sb = pB.tile([128, 4 * 256], BF16, tag="wkb", name=nm("wkb"))
        nc.gpsimd.dma_start(out=wkb_sb[:],
                            in_=w_kbT.rearrange("(k p) e -> p k e", p=128))
        wvb_sb = pB.tile([128, 4 * 256], BF16, tag="wvb", name=nm("wvb"))
        nc.gpsimd.dma_start(out=wvb_sb[:],
                            in_=w_vbT.rearrange("(k p) e -> p k e", p=128))
        wqb_sb = pB.tile([128, 12 * 384], BF16, tag="wqb", name=nm("wqb"))
        nc.gpsimd.dma_start(out=wqb_sb[:],
                            in_=w_qbT.rearrange("(k p) e -> p k e", p=128))
        mask_sb = pB.tile([128, 16 * 128], BF16, tag="mask", name=nm("mask"))
        nc.gpsimd.dma_start(out=mask_sb[:],
                            in_=masks.rearrange("i p f -> p i f"))
        wo_sb = pB.tile([128, 16 * EC], BF16, tag="wo", name=nm("wo"))
        nc.gpsimd.dma_start(out=wo_sb[:],
                            in_=w_oT.rearrange("(k p) e -> p k e", p=128))

        def wkb(k):
            return wkb_sb[:, 256 * k:256 * (k + 1)]

        def wvb(k):
            return wvb_sb[:, 256 * k:256 * (k + 1)]

        def wqb(k):
            return wqb_sb[:, 384 * k:384 * (k + 1)]

        # batched loads of the gathered activations (feature-major)
        agkv3 = agkv_out.rearrange("(c r) t -> r c t", c=NC)   # [576, 8, 256]
        kv_sb = []
        for i in range(4):
            t = pB.tile([128, T], BF16, tag=f"kv{i}", name=nm(f"kv{i}"))
            nc.scalar.dma_start(out=t[:], in_=agkv3[128 * i:128 * (i + 1)])
            kv_sb.append(t)
        kpe_sb = pB.tile([64, T], BF16, tag="kpe", name=nm("kpe"))
        nc.scalar.dma_start(out=kpe_sb[:], in_=agkv3[KVL:KVR])
        agq3 = agq_out.rearrange("(c r) t -> r c t", c=NC)   # [1536, 8, 256]
        qA = []
        for k in range(12):
            t = pB.tile([128, T], FP8, tag=f"qA{k}", name=nm(f"qA{k}"))
            eng = nc.scalar if k % 2 == 0 else nc.sync
            eng.dma_start(out=t[:], in_=agq3[128 * k:128 * (k + 1)])
            qA.append(t)

        qn_sb = [pB.tile([128, T], BF16, tag=f"qn{h}", name=nm(f"qn{h}"))
                 for h in range(HC)]
        qp_sb = [pB.tile([64, T], BF16, tag=f"qp{h}", name=nm(f"qp{h}"))
                 for h in range(HC)]
        kn_sb = [pB.tile([128, T], BF16, tag=f"kn{h}", name=nm(f"kn{h}"))
                 for h in range(HC)]
        v_sb = [pB.tile([128, 2 * DV], BF16, tag=f"v{i}", name=nm(f"v{i}"))
                for i in range(16)]

        with tc.tile_pool(name="psB", bufs=1, space="PSUM") as psB:
            # ============ B2: kv_b expansion (overlaps the q AllGather) ====
            for h in range(HC):
                for jp in range(2):           # j-pairs, lhsT reused across js
                    kn_ps = [psB.tile([128, 512], F32, tag="b2", bufs=2,
                                      name=nm("kn_ps")) for _ in range(2)]
                    for i in range(4):
                        for jj in range(2):
                            j = 2 * jp + jj
                            nc.tensor.matmul(
                                kn_ps[jj][:], wkb(i)[:, 128 * h:128 * (h + 1)],
                                kv_sb[i][:, 512 * j:512 * (j + 1)],
                                start=(i == 0), stop=(i == 3))
                    for jj in range(2):
                        j = 2 * jp + jj
                        nc.vector.tensor_copy(
                            kn_sb[h][:, 512 * j:512 * (j + 1)], kn_ps[jj][:])
            for s16 in range(16):
                v_ps = psB.tile([128, 2 * DV], F32, tag="b2", bufs=2,
                                name=nm("v_ps"))
                for i in range(4):
                    nc.tensor.matmul(v_ps[:],
                                     kv_sb[i][:, 128 * s16:128 * (s16 + 1)],
                                     wvb(i)[:], start=(i == 0), stop=(i == 3))
                nc.vector.tensor_copy(v_sb[s16][:], v_ps[:])

            # ============ B1: q_b projection + q rope ======================
            for js in ((0, 1), (2, 3)):
                qn_ps = {}
                qp_ps = {}
                for j in js:
                    qn_ps[(j, 0)] = psB.tile([128, 512], F32, tag="b1qn",
                                             bufs=4, name=nm("qn_ps"))
                    qn_ps[(j, 1)] = psB.tile([128, 512], F32, tag="b1qn",
                                             bufs=4, name=nm("qn_ps"))
                    qp_ps[j] = psB.tile([128, 512], F32, tag="b1qp",
                                        bufs=2, name=nm("qp_ps"))
                for k in range(12):
                    for m in range(3):        # stationary reused across js
                        for j in js:
                            dst = qp_ps[j] if m == 2 else qn_ps[(j, m)]
                            nc.tensor.matmul(
                                dst[:], wqb(k)[:, 128 * m:128 * (m + 1)],
                                qA[k][:, 512 * j:512 * (j + 1)],
                                start=(k == 0), stop=(k == 11))
                for j in js:
                    cs = cos_sb[:, 512 * j:512 * (j + 1)]
                    sn = sin_sb[:, 512 * j:512 * (j + 1)]
                    for h in range(HC):
                        nc.scalar.copy(
                            qn_sb[h][:, 512 * j:512 * (j + 1)],
                            qn_ps[(j, h)][:])
                        base = 64 * h
                        a1 = sp.tile([32, 512], F32, tag="ro1", bufs=2,
                                     name=nm("ro1"))
                        a2 = sp.tile([32, 512], F32, tag="ro2", bufs=2,
                                     name=nm("ro2"))
                        nc.vector.tensor_mul(
                            a1[:], qp_ps[j][base:base + 32, :], cs)
                        nc.vector.tensor_mul(
                            a2[:], qp_ps[j][base + 32:base + 64, :], sn)
                        nc.vector.tensor_sub(
                            qp_sb[h][0:32, 512 * j:512 * (j + 1)], a1[:], a2[:])
                        a3 = sp.tile([32, 512], F32, tag="ro1", bufs=2,
                                     name=nm("ro1"))
                        a4 = sp.tile([32, 512], F32, tag="ro2", bufs=2,
                                     name=nm("ro2"))
                        nc.vector.tensor_mul(
                            a3[:], qp_ps[j][base + 32:base + 64, :], cs)
                        nc.vector.tensor_mul(
                            a4[:], qp_ps[j][base:base + 32, :], sn)
                        nc.vector.tensor_add(
                            qp_sb[h][32:64, 512 * j:512 * (j + 1)], a3[:], a4[:])

        # ============ B3: attention, two phases of two interleaved js ======
        PHASES = [(0, (0, 1), 8), (1, (2, 3), 16)]
        COL_OF_J = {0: 0, 1: 512, 2: 0, 3: 512}
        with tc.tile_pool(name="psC", bufs=1, space="PSUM") as psC:
            for half, js, imax in PHASES:
                for h in range(HC):
                    num_ps = {j: psC.tile([128, 512], F32, tag="acc", bufs=4,
                                          name=nm("num")) for j in js}
                    dacc = {j: sp.tile([128, 512], F32, tag="dacc", bufs=4,
                                       name=nm("dacc")) for j in js}

                    def finalize(j):
                        db = sp.tile([128, 512], BF16, tag="daccb", bufs=2,
                                     name=nm("daccb"))
                        nc.vector.tensor_copy(db[:], dacc[j][:])
                        den_ps = psC.tile([1, 512], F32, tag="sc", bufs=4,
                                          name=nm("den"))
                        nc.tensor.matmul(den_ps[:], ones_bf[:], db[:],
                                         start=True, stop=True)
                        rec = sp.tile([1, 512], F32, tag="rec", bufs=2,
                                      name=nm("rec"))
                        nc.vector.reciprocal_approx_fast(rec[:], den_ps[:])
                        bc = sp.tile([128, 512], F32, tag="bc", bufs=2,
                                     name=nm("bc"))
                        nc.gpsimd.partition_broadcast(bc[:], rec[:])
                        ao = sp.tile([128, 512], BF16, tag="ao", bufs=2,
                                     name=nm("ao"))
                        nc.vector.tensor_mul(ao[:], num_ps[j][:], bc[:])
                        if half == 0:
                            nc.sync.dma_start(
                                out=ag2_in[DV * h:DV * (h + 1),
                                           COL_OF_J[j]:COL_OF_J[j] + 512],
                                in_=ao[:])
                        else:
                            nc.sync.dma_start(
                                out=ag2j_in[j][DV * h:DV * (h + 1), :],
                                in_=ao[:])
                            if h == HC - 1 and j == 2:
                                # j2 done on both heads: fire its gather early
                                nc.gpsimd.collective_compute(
                                    "AllGather", mybir.AluOpType.bypass,
                                    replica_groups=rg,
                                    ins=[ag2j_in[2][:]], outs=[ag2j_out[2][:]])

                    pend = []

                    def flush():
                        for (p_sb, j, ii, c0) in pend:
                            nc.tensor.matmul(
                                num_ps[j][:, c0:512],
                                v_sb[ii][:, 128 * h:128 * (h + 1)],
                                p_sb[:], start=(ii == 0),
                                stop=(ii == 4 * j + 3))
                        for (p_sb, j, ii, c0) in pend:
                            if ii == 4 * j + 3:
                                finalize(j)
                        pend.clear()

                    for i in range(imax):
                        cur = [j for j in js if i <= 4 * j + 3]
                        scs = {}
                        for j in cur:
                            q0 = i - 4 * j
                            col0 = 128 * q0 if q0 > 0 else 0
                            sc = psC.tile([128, 512 - col0], F32, tag="sc",
                                          bufs=4, name=nm("sc"))
                            scs[j] = (sc, col0)
                        for j in cur:
                            sc, col0 = scs[j]
                            nc.tensor.matmul(
                                sc[:], kn_sb[h][:, 128 * i:128 * (i + 1)],
                                qn_sb[h][:, 512 * j + col0:512 * (j + 1)],
                                start=True, stop=False)
                        for j in cur:
                            sc, col0 = scs[j]
                            nc.tensor.matmul(
                                sc[:], kpe_sb[:, 128 * i:128 * (i + 1)],
                                qp_sb[h][:, 512 * j + col0:512 * (j + 1)],
                                start=False, stop=True)
                        nxt = []
                        for j in cur:
                            sc, col0 = scs[j]
                            w = 512 - col0
                            p_sb = sp.tile([128, w], BF16, tag="p", bufs=6,
                                           name=nm("p"))
                            nc.scalar.activation(
                                p_sb[:], sc[:],
                                mybir.ActivationFunctionType.Exp, scale=SCALE)
                            if i >= 4 * j:
                                nc.vector.tensor_mul(
                                    p_sb[:, 0:128], p_sb[:, 0:128],
                                    mask_sb[:, 128 * i:128 * (i + 1)])
                            deng = nc.vector if j % 2 else nc.gpsimd
                            if i == 0:
                                deng.tensor_copy(dacc[j][:], p_sb[:])
                            else:
                                deng.tensor_add(
                                    dacc[j][:, col0:512],
                                    dacc[j][:, col0:512], p_sb[:])
                            nxt.append((p_sb, j, i, col0))
                        flush()
                        pend.extend(nxt)
                    flush()

                if half == 0:
                    nc.gpsimd.collective_compute(
                        "AllGather", mybir.AluOpType.bypass, replica_groups=rg,
                        ins=[ag2_in[:]], outs=[ag2_out[:]])
                else:
                    nc.gpsimd.collective_compute(
                        "AllGather", mybir.AluOpType.bypass, replica_groups=rg,
                        ins=[ag2j_in[3][:]], outs=[ag2j_out[3][:]])

            # ============ B5: o_proj (feature-sharded) =====================
            # half 0: combined [*, 1024] gather; half 1: per-j gathers.
            def oproj(src, jcols):
                njl = len(jcols)
                ops = {}
                for me in range(2):
                    for jl in range(njl):
                        ops[(me, jl)] = psC.tile([128, 512], F32, tag="acc",
                                                 bufs=4, name=nm("op"))
                for k in range(16):
                    rhs = sp.tile([128, 512 * njl], BF16, tag="orhs", bufs=4,
                                  name=nm("orhs"))
                    eng = nc.scalar if k % 2 == 0 else nc.sync
                    eng.dma_start(out=rhs[:],
                                  in_=src[128 * k:128 * (k + 1), :])
                    for me in range(2):
                        for jl in range(njl):
                            nc.tensor.matmul(
                                ops[(me, jl)][:],
                                wo_sb[:, EC * k + 128 * me:EC * k + 128 * (me + 1)],
                                rhs[:, 512 * jl:512 * (jl + 1)],
                                start=(k == 0), stop=(k == 15))
                for jl in range(njl):
                    j = jcols[jl]
                    for me in range(2):
                        yo = sp.tile([128, 512], F32, tag="yo", bufs=2,
                                     name=nm("yo"))
                        nc.vector.tensor_copy(yo[:], ops[(me, jl)][:])
                        nc.sync.dma_start(
                            out=y_out[128 * me:128 * (me + 1),
                                      512 * j:512 * (j + 1)],
                            in_=yo[:])

            oproj(ag2_out, (0, 1))
            oproj(ag2j_out[2], (2,))
            oproj(ag2j_out[3], (3,))

    nc.compile()
    return nc


_PROGRAM = None


def _get_program():
    global _PROGRAM
    if _PROGRAM is None:
        _PROGRAM = build_program()
    return _PROGRAM


def _prep_inputs(positions, hidden_states, w_q_a, q_a_ln_w, w_q_b, w_kv_a,
                 kv_a_ln_w, w_kv_b, w_o):
    pos = np.asarray(positions).astype(np.float32)
    hidden_states = np.asarray(hidden_states, dtype=np.float32)
    w_q_a = np.asarray(w_q_a, dtype=np.float32)
    q_a_ln_w = np.asarray(q_a_ln_w, dtype=np.float32)
    w_q_b = np.asarray(w_q_b, dtype=np.float32)
    w_kv_a = np.asarray(w_kv_a, dtype=np.float32)
    kv_a_ln_w = np.asarray(kv_a_ln_w, dtype=np.float32)
    w_kv_b = np.asarray(w_kv_b, dtype=np.float32)
    w_o = np.asarray(w_o, dtype=np.float32)

    perm = np.concatenate([np.arange(0, DR, 2), np.arange(1, DR, 2)])

    inv = 1.0 / (THETA ** (np.arange(0, DR, 2, dtype=np.float32) / DR))
    f = pos[:, None] * inv[None, :]                      # [T, 32]
    cos_t = np.ascontiguousarray(np.cos(f).astype(np.float32).T)
    sin_t = np.ascontiguousarray(np.sin(f).astype(np.float32).T)

    # diag-block keep masks: for s-tile i, t-super i//4
    masks = np.zeros((16, 128, 128), dtype=NPBF16)
    for i in range(16):
        s_idx = pos[128 * i:128 * (i + 1)]
        masks[i] = (s_idx[None, :] >= s_idx[:, None]).astype(NPBF16)

    # pack (J,k) tiles contiguous: row ((16J+k)*128+p) = w_q_a.T[128k+p, 512J+e]
    w_q_aT = np.ascontiguousarray(
        w_q_a.T.reshape(16, 128, 3, 512).transpose(2, 0, 1, 3)
        .reshape(48 * 128, 512)).astype(NPBF16)
    w_kv_aT = np.ascontiguousarray(w_kv_a.T)                     # [2048, 576]
    w_kv_aT[:, KVL:] = w_kv_aT[:, KVL:][:, perm]
    w_kv_aT = w_kv_aT.astype(NPBF16)

    hid_T_full = np.ascontiguousarray(hidden_states.T).astype(NPBF16)

    in_maps = []
    for c in range(NC):
        h0, h1 = 2 * c, 2 * c + 1
        Wh0 = w_q_b[h0 * (DN + DR):(h0 + 1) * (DN + DR), :]      # [192, 1536]
        Wh1 = w_q_b[h1 * (DN + DR):(h1 + 1) * (DN + DR), :]
        blocks = [Wh0[:DN], Wh1[:DN], Wh0[DN:][perm], Wh1[DN:][perm]]
        w_qbT = (np.concatenate(blocks, axis=0).T
                 * q_a_ln_w[:, None]).astype(NPBF16)             # [1536, 384]
        w_kbT = (np.concatenate(
            [w_kv_b[h * (DN + DV):h * (DN + DV) + DN] for h in (h0, h1)],
            axis=0).T * kv_a_ln_w[:, None]).astype(NPBF16)       # [512, 256]
        w_vbT = (np.concatenate(
            [w_kv_b[h * (DN + DV) + DN:(h + 1) * (DN + DV)] for h in (h0, h1)],
            axis=0).T * kv_a_ln_w[:, None]).astype(NPBF16)       # [512, 256]
        w_oTc = np.ascontiguousarray(w_o.T[:, EC * c:EC * (c + 1)]).astype(NPBF16)
        # hid chunk, rows permuted p-major: dram row (16 p + k) = orig 128 k + p
        X = hid_T_full[:, TC * c:TC * (c + 1)]
        hid_pm = np.ascontiguousarray(
            X.reshape(16, 128, TC).transpose(1, 0, 2).reshape(HID, TC))
        in_maps.append({
            "hid_T": hid_pm,
            "w_q_aT": w_q_aT,
            "w_kv_aT": w_kv_aT,
            "w_qbT": np.ascontiguousarray(w_qbT),
            "w_kbT": np.ascontiguousarray(w_kbT),
            "w_vbT": np.ascontiguousarray(w_vbT),
            "w_oT": w_oTc,
            "cos_t": cos_t,
            "sin_t": sin_t,
            "cos_c": np.ascontiguousarray(cos_t[:, TC * c:TC * (c + 1)]),
            "sin_c": np.ascontiguousarray(sin_t[:, TC * c:TC * (c + 1)]),
            "masks": masks,
        })
    return in_maps


RUN_KWARGS = {}
LAST_RESULT = None


def kernel(**inputs):
    global LAST_RESULT
    nc = _get_program()
    in_maps = _prep_inputs(**inputs)
    res = run_bass_kernel_spmd(nc, in_maps, list(range(NC)), **RUN_KWARGS)
    LAST_RESULT = res
    yT = np.concatenate([res.results[c]["y_out"] for c in range(NC)], axis=0)
    return np.ascontiguousarray(yT.T)


# revision 30
# speedup vs baseline: 1.2130x; 1.2130x over previous
"""DeepseekV2 MLA attention fusion (prefill, T=2048) on 8 Trainium2 cores.

Sharding:
  - Phase A (q_a / kv_a projections + rmsnorm + k_pe rope): sharded over tokens
    (256 tokens/core), feature-major outputs.  kv/k_pe AllGathered early
    (bf16), q_c AllGathered in two row-halves as fp8e4 (the rmsnormed q_c is
    O(1), e4m3 transport keeps rel err ~1e-2) so the dominant collective is
    half the bytes.
  - Phase B tensor-parallel over heads (2 heads/core).  kv_b expansion (B2)
    overlaps the q AllGather; q_b projection (B1) consumes the fp8 q_c
    directly as the matmul moving operand, with a packed rope stationary
    (3 matmuls/k-tile for 384 output cols) and natural k-split accumulation
    so its first half starts as soon as agq half-a lands.
  - Attention runs per 512-token j-supertile in DESCENDING order (j3..j0);
    each j's normalized output is AllGathered (bf16) immediately and its
    o_proj (feature-sharded, 256 cols/core) queues behind the remaining
    attention work, so the final gather overlaps queued o_proj matmuls.
  - softmax denominator: fp32 vector accumulation of the exp tiles + a single
    ones-matmul per (j,h) instead of a PE matmul per s-tile.

All matmuls bf16 (q_b: bf16 stationary x fp8 moving) with fp32 PSUM
accumulation; softmax in fp32.
"""
import sys

sys.path.insert(0, "/opt/trn_rl_repo")

from contextlib import ExitStack

import ml_dtypes
import numpy as np

from concourse import bacc, bass, mybir, tile
from concourse.bass_utils import run_bass_kernel_spmd

BF16 = mybir.dt.bfloat16
F32 = mybir.dt.float32
FP8 = mybir.dt.float8e4
NPBF16 = ml_dtypes.bfloat16
NPFP8 = ml_dtypes.float8_e4m3

T = 2048
HID = 2048
H = 16
DN = 128
DR = 64
DV = 128
QL = 1536
KVL = 512
KVR = KVL + DR
THETA = 10000.0
EPS = 1e-6
SCALE = (DN + DR) ** -0.5

NC = 8            # cores
TC = T // NC      # tokens per core = 256
HC = H // NC      # heads per core = 2
EC = HID // NC    # o_proj output cols per core = 256


def build_program():
    nc = bacc.Bacc("TRN2", target_bir_lowering=False, debug=False, num_devices=NC)

    _ctr = [0]

    def nm(tag):
        _ctr[0] += 1
        return f"t{_ctr[0]}_{tag}"

    def din(name, shape, dt=BF16):
        return nc.dram_tensor(name, shape, dt, kind="ExternalInput").ap()

    hid_T = din("hid_T", [HID, TC])                 # hidden.T chunk, p-major rows
    w_q_aT = din("w_q_aT", [48 * 128, 512])         # packed (J,k) tiles
    w_kv_aT = din("w_kv_aT", [HID, KVR])            # rope cols de-interleaved
    w_qbT = din("w_qbT", [QL, 4 * 128])             # [h0n, h1n, rope, rope_sw]
    w_kbT = din("w_kbT", [KVL, 2 * DN])             # [h0 k, h1 k]
    w_vbT = din("w_vbT", [KVL, 2 * DV])             # [h0 v, h1 v]
    w_oT = din("w_oT", [H * DV, EC])
    csf_t = din("csf_t", [128, T])                  # cos x4 groups (bf16)
    snf_t = din("snf_t", [128, T])                  # [-sin,sin,-sin,sin]
    cos_c = din("cos_c", [DR // 2, TC], F32)        # core's chunk (k rope)
    sin_c = din("sin_c", [DR // 2, TC], F32)
    masks = din("masks", [16, 128, 128])            # diag triangles

    y_out = nc.dram_tensor("y_out", [EC, T], F32, kind="ExternalOutput").ap()

    agkv_in = nc.dram_tensor("agkv_in", [KVR, TC], BF16).ap()
    agkv_out = nc.dram_tensor("agkv_out", [NC * KVR, TC], BF16,
                              addr_space="Shared").ap()
    agq_in = nc.dram_tensor("agq_in", [QL, TC], FP8).ap()
    agq_out = nc.dram_tensor("agq_out", [NC * QL, TC], FP8,
                             addr_space="Shared").ap()
    ag2_in = nc.dram_tensor("ag2a_in", [HC * DV, 1024], BF16).ap()
    ag2_out = nc.dram_tensor("ag2a_out", [H * DV, 1024], BF16,
                             addr_space="Shared").ap()
    ag2j_in = {j: nc.dram_tensor(f"ag2{j}_in", [HC * DV, 512], BF16).ap()
               for j in (2, 3)}
    ag2j_out = {j: nc.dram_tensor(f"ag2{j}_out", [H * DV, 512], BF16,
                                  addr_space="Shared").ap() for j in (2, 3)}

    rg = [list(range(NC))]

    with tile.TileContext(nc) as tc, ExitStack() as ctx:
        const = ctx.enter_context(tc.tile_pool(name="const", bufs=1))
        sp = ctx.enter_context(tc.tile_pool(name="sp", bufs=2))

        ones_bf = const.tile([128, 1], BF16, tag="ones_bf", name=nm("ones_bf"))
        nc.vector.memset(ones_bf[:], 1.0)

        # ============ Phase A (token-sharded, feature-major outputs) =======
        ones_col = const.tile([128, 1], F32, tag="ones_col", name=nm("ones_col"))
        nc.vector.memset(ones_col[:], 1.0)
        eps1_sb = const.tile([1, 1], F32, tag="eps1", name=nm("eps1"))
        nc.vector.memset(eps1_sb[:], EPS)

        with tc.tile_pool(name="pA", bufs=1) as pA, \
             tc.tile_pool(name="psA", bufs=1, space="PSUM") as psA:
            # rows pre-permuted host-side: dram row (16 p + k) = contraction
            # row (128 k + p), so each chunk is contiguous per partition.
            hid4 = hid_T.rearrange("(p g k) t -> p g (k t)", p=128, g=4)
            hid_sb = []
            for g in range(4):
                t = pA.tile([128, 4 * TC], BF16, tag=f"hid{g}",
                            name=nm(f"hid{g}"))
                nc.sync.dma_start(out=t[:], in_=hid4[:, g, :])
                hid_sb.append(t)

            def hid_k(k):
                g, kk = divmod(k, 4)
                return hid_sb[g][:, TC * kk:TC * (kk + 1)]

            def norm_scale(sumsq, n_feat):
                """rsqrt(mean+eps) broadcast to [128, TC] SBUF tile."""
                s = sp.tile([1, TC], F32, tag="nrm", bufs=2, name=nm("nrm"))
                nc.scalar.activation(s[:], sumsq[:],
                                     mybir.ActivationFunctionType.Sqrt,
                                     bias=eps1_sb[:], scale=1.0 / n_feat)
                r = sp.tile([1, TC], F32, tag="nrm_r", bufs=2, name=nm("nrm_r"))
                nc.vector.reciprocal_approx_fast(r[:], s[:])
                bc = sp.tile([128, TC], F32, tag="nrm_bc", bufs=2,
                             name=nm("nrm_bc"))
                nc.gpsimd.partition_broadcast(bc[:], r[:])
                return bc

            # ---- latent: kv_c 4 blocks + k_pe, fused per-k loop ----
            kv_ps = [psA.tile([128, TC], F32, tag="a_ps", bufs=4,
                              name=nm("a_ps")) for _ in range(4)]
            pe_ps = psA.tile([64, TC], F32, tag="pe_ps", bufs=1, name=nm("pe_ps"))
            for k in range(16):
                wt = sp.tile([128, KVR], BF16, tag="wA", bufs=4, name=nm("wA"))
                nc.scalar.dma_start(out=wt[:],
                                    in_=w_kv_aT[128 * k:128 * (k + 1), :])
                for jj in range(4):
                    nc.tensor.matmul(kv_ps[jj][:], wt[:, 128 * jj:128 * (jj + 1)],
                                     hid_k(k), start=(k == 0), stop=(k == 15))
                nc.tensor.matmul(pe_ps[:], wt[:, KVL:KVR], hid_k(k),
                                 start=(k == 0), stop=(k == 15))
            kv_sq = psA.tile([1, TC], F32, tag="sumsq", bufs=2, name=nm("sumsq"))
            kvc_sb = []
            for jj in range(4):
                sq = sp.tile([128, TC], F32, tag="sq", bufs=2, name=nm("sq"))
                nc.scalar.square(sq[:], kv_ps[jj][:])
                o = pA.tile([128, TC], F32, tag=f"kvc{jj}", name=nm("kvc"))
                nc.vector.tensor_copy(o[:], kv_ps[jj][:])
                kvc_sb.append(o)
                nc.tensor.matmul(kv_sq[:], ones_col[:], sq[:],
                                 start=(jj == 0), stop=(jj == 3))
            kbc = norm_scale(kv_sq, KVL)
            for jj in range(4):
                o = sp.tile([128, TC], BF16, tag="agst", bufs=3, name=nm("agst"))
                nc.vector.tensor_mul(o[:], kvc_sb[jj][:], kbc[:])
                nc.sync.dma_start(out=agkv_in[128 * jj:128 * (jj + 1), :],
                                  in_=o[:])
            # k_pe rope (partition pairs 0:32 / 32:64, de-interleaved)
            cosc = pA.tile([32, TC], F32, tag="cosc", name=nm("cosc"))
            sinc = pA.tile([32, TC], F32, tag="sinc", name=nm("sinc"))
            nc.sync.dma_start(out=cosc[:], in_=cos_c[:])
            nc.sync.dma_start(out=sinc[:], in_=sin_c[:])
            kpe_ro = sp.tile([64, TC], BF16, tag="kpero", bufs=1, name=nm("kpero"))
            t1 = sp.tile([32, TC], F32, tag="ro1", bufs=2, name=nm("ro1"))
            t2 = sp.tile([32, TC], F32, tag="ro2", bufs=2, name=nm("ro2"))
            nc.vector.tensor_mul(t1[:], pe_ps[0:32, :], cosc[:])
            nc.vector.tensor_mul(t2[:], pe_ps[32:64, :], sinc[:])
            nc.vector.tensor_sub(kpe_ro[0:32, :], t1[:], t2[:])
            t3 = sp.tile([32, TC], F32, tag="ro1", bufs=2, name=nm("ro1"))
            t4 = sp.tile([32, TC], F32, tag="ro2", bufs=2, name=nm("ro2"))
            nc.vector.tensor_mul(t3[:], pe_ps[32:64, :], cosc[:])
            nc.vector.tensor_mul(t4[:], pe_ps[0:32, :], sinc[:])
            nc.vector.tensor_add(kpe_ro[32:64, :], t3[:], t4[:])
            nc.sync.dma_start(out=agkv_in[KVL:KVR, :], in_=kpe_ro[:])

            nc.gpsimd.collective_compute(
                "AllGather", mybir.AluOpType.bypass, replica_groups=rg,
                ins=[agkv_in[:]], outs=[agkv_out[:]])

            # ---- q_c: 12 blocks in 3 J-supertiles; fp8 transport ----
            q_sq = psA.tile([1, TC], F32, tag="sumsq", bufs=2, name=nm("sumsq"))
            qc_sb = []
            for J in range(3):
                pss = [psA.tile([128, TC], F32, tag="a_ps", bufs=4,
                                name=nm("a_ps")) for _ in range(4)]
                for k in range(16):
                    wt = sp.tile([128, 512], BF16, tag="wAq", bufs=4,
                                 name=nm("wAq"))
                    r0 = (16 * J + k) * 128
                    nc.sync.dma_start(out=wt[:], in_=w_q_aT[r0:r0 + 128, :])
                    for jj in range(4):
                        nc.tensor.matmul(pss[jj][:],
                                         wt[:, 128 * jj:128 * (jj + 1)],
                                         hid_k(k), start=(k == 0), stop=(k == 15))
                for jj in range(4):
                    j = 4 * J + jj
                    sq = sp.tile([128, TC], F32, tag="sq", bufs=2, name=nm("sq"))
                    nc.scalar.square(sq[:], pss[jj][:])
                    o = pA.tile([128, TC], F32, tag=f"qc{j}", name=nm("qc"))
                    nc.vector.tensor_copy(o[:], pss[jj][:])
                    qc_sb.append(o)
                    nc.tensor.matmul(q_sq[:], ones_col[:], sq[:],
                                     start=(j == 0), stop=(j == 11))
            qbc = norm_scale(q_sq, QL)
            for j in range(12):
                o = sp.tile([128, TC], FP8, tag="agstq", bufs=3,
                            name=nm("agstq"))
                nc.vector.tensor_mul(o[:], qc_sb[j][:], qbc[:])
                nc.sync.dma_start(
                    out=agq_in[128 * j:128 * (j + 1), :], in_=o[:])
            nc.gpsimd.collective_compute(
                "AllGather", mybir.AluOpType.bypass, replica_groups=rg,
                ins=[agq_in[:]], outs=[agq_out[:]])

        # ============ Phase B resident tensors =============================
        pB = ctx.enter_context(tc.tile_pool(name="pB", bufs=1))
        csf_sb = pB.tile([128, T], BF16, tag="csf", name=nm("csf"))
        snf_sb = pB.tile([128, T], BF16, tag="snf", name=nm("snf"))
        nc.sync.dma_start(out=csf_sb[:], in_=csf_t[:])
        nc.sync.dma_start(out=snf_sb[:], in_=snf_t[:])

        wkb_sb = pB.tile([128, 4 * 256], BF16, tag="wkb", name=nm("wkb"))
        nc.sync.dma_start(out=wkb_sb[:],
                          in_=w_kbT.rearrange("(k p) e -> p k e", p=128))
        wvb_sb = pB.tile([128, 4 * 256], BF16, tag="wvb", name=nm("wvb"))
        nc.sync.dma_start(out=wvb_sb[:],
                          in_=w_vbT.rearrange("(k p) e -> p k e", p=128))
        wqb_sb = pB.tile([128, 12 * 512], BF16, tag="wqb", name=nm("wqb"))
        nc.sync.dma_start(out=wqb_sb[:],
                          in_=w_qbT.rearrange("(k p) e -> p k e", p=128))
        mask_sb = pB.tile([128, 16 * 128], BF16, tag="mask", name=nm("mask"))
        nc.sync.dma_start(out=mask_sb[:],
                          in_=masks.rearrange("i p f -> p i f"))
        wo_sb = pB.tile([128, 16 * EC], BF16, tag="wo", name=nm("wo"))
        nc.sync.dma_start(out=wo_sb[:],
                          in_=w_oT.rearrange("(k p) e -> p k e", p=128))

        def wkb(k):
            return wkb_sb[:, 256 * k:256 * (k + 1)]

        def wvb(k):
            return wvb_sb[:, 256 * k:256 * (k + 1)]

        def wqb(k):
            return wqb_sb[:, 512 * k:512 * (k + 1)]

        # batched loads of the gathered activations (feature-major)
        agkv3 = agkv_out.rearrange("(c r) t -> r c t", c=NC)   # [576, 8, 256]
        kv_sb = []
        for i in range(4):
            t = pB.tile([128, T], BF16, tag=f"kv{i}", name=nm(f"kv{i}"))
            nc.scalar.dma_start(out=t[:], in_=agkv3[128 * i:128 * (i + 1)])
            kv_sb.append(t)
        kpe_sb = pB.tile([64, T], BF16, tag="kpe", name=nm("kpe"))
        nc.scalar.dma_start(out=kpe_sb[:], in_=agkv3[KVL:KVR])
        agq3 = agq_out.rearrange("(c r) t -> r c t", c=NC)   # [1536, 8, 256]
        qA = []
        for k in range(12):
            t = pB.tile([128, T], FP8, tag=f"qA{k}", name=nm(f"qA{k}"))
            nc.scalar.dma_start(out=t[:], in_=agq3[128 * k:128 * (k + 1)])
            qA.append(t)

        qn_sb = [pB.tile([128, T], BF16, tag=f"qn{h}", name=nm(f"qn{h}"))
                 for h in range(HC)]
        qp_sb = [pB.tile([64, T], BF16, tag=f"qp{h}", name=nm(f"qp{h}"))
                 for h in range(HC)]
        kn_sb = [pB.tile([128, T], BF16, tag=f"kn{h}", name=nm(f"kn{h}"))
                 for h in range(HC)]
        v_sb = [pB.tile([128, 2 * DV], BF16, tag=f"v{i}", name=nm(f"v{i}"))
                for i in range(16)]

        with tc.tile_pool(name="psB2", bufs=1, space="PSUM") as psB:
            # ============ B2: kv_b expansion (overlaps the q AllGather) ====
            for h in range(HC):
                for jp in range(2):           # j-pairs, lhsT reused across js
                    kn_ps = [psB.tile([128, 512], F32, tag="b2", bufs=2,
                                      name=nm("kn_ps")) for _ in range(2)]
                    for i in range(4):
                        for jj in range(2):
                            j = 2 * jp + jj
                            nc.tensor.matmul(
                                kn_ps[jj][:], wkb(i)[:, 128 * h:128 * (h + 1)],
                                kv_sb[i][:, 512 * j:512 * (j + 1)],
                                start=(i == 0), stop=(i == 3))
                    for jj in range(2):
                        j = 2 * jp + jj
                        nc.vector.tensor_copy(
                            kn_sb[h][:, 512 * j:512 * (j + 1)], kn_ps[jj][:])
            for s16 in range(16):
                v_ps = psB.tile([128, 2 * DV], F32, tag="b2", bufs=2,
                                name=nm("v_ps"))
                for i in range(4):
                    nc.tensor.matmul(v_ps[:],
                                     kv_sb[i][:, 128 * s16:128 * (s16 + 1)],
                                     wvb(i)[:], start=(i == 0), stop=(i == 3))
                nc.vector.tensor_copy(v_sb[s16][:], v_ps[:])

        # ============ B1: q_b projection + q rope ======================
        with tc.tile_pool(name="psB1", bufs=1, space="PSUM") as psB1:
            for js in ((0, 1), (2, 3)):
                qn_ps = {}
                qp_ps = {}
                qps_ps = {}
                for j in js:
                    qn_ps[(j, 0)] = psB1.tile([128, 512], F32, tag="b1qn",
                                              bufs=4, name=nm("qn_ps"))
                    qn_ps[(j, 1)] = psB1.tile([128, 512], F32, tag="b1qn",
                                              bufs=4, name=nm("qn_ps"))
                    qp_ps[j] = psB1.tile([128, 512], F32, tag="b1qp",
                                         bufs=2, name=nm("qp_ps"))
                    qps_ps[j] = psB1.tile([128, 512], F32, tag="b1qs",
                                          bufs=2, name=nm("qps_ps"))
                for k in range(12):
                    for m in range(4):        # stationary reused across js
                        for j in js:
                            dst = (qn_ps[(j, 0)], qn_ps[(j, 1)],
                                   qp_ps[j], qps_ps[j])[m]
                            nc.tensor.matmul(
                                dst[:], wqb(k)[:, 128 * m:128 * (m + 1)],
                                qA[k][:, 512 * j:512 * (j + 1)],
                                start=(k == 0), stop=(k == 11))
                for j in js:
                    for h in range(HC):
                        nc.scalar.copy(
                            qn_sb[h][:, 512 * j:512 * (j + 1)],
                            qn_ps[(j, h)][:])
                    # rope: qp2 = qp * cos4 + qp_swapped * (+-sin)
                    t1 = sp.tile([128, 512], F32, tag="ro1", bufs=2,
                                 name=nm("ro1"))
                    t2 = sp.tile([128, 512], F32, tag="ro2", bufs=2,
                                 name=nm("ro2"))
                    nc.vector.tensor_mul(t1[:], qp_ps[j][:],
                                         csf_sb[:, 512 * j:512 * (j + 1)])
                    nc.vector.tensor_mul(t2[:], qps_ps[j][:],
                                         snf_sb[:, 512 * j:512 * (j + 1)])
                    for h in range(HC):
                        nc.vector.tensor_add(
                            qp_sb[h][:, 512 * j:512 * (j + 1)],
                            t1[64 * h:64 * (h + 1), :],
                            t2[64 * h:64 * (h + 1), :])

        # ============ B3: attention, two phases of two interleaved js ======
        PHASES = [(0, (0, 1), 8), (1, (2, 3), 16)]
        COL_OF_J = {0: 0, 1: 512, 2: 0, 3: 512}
        with tc.tile_pool(name="psC", bufs=1, space="PSUM") as psC:
            for half, js, imax in PHASES:
                for h in range(HC):
                    num_ps = {j: psC.tile([128, 512], F32, tag="acc", bufs=4,
                                          name=nm("num")) for j in js}
                    dacc = {j: sp.tile([128, 512], F32, tag="dacc", bufs=4,
                                       name=nm("dacc")) for j in js}

                    def finalize(j):
                        db = sp.tile([128, 512], BF16, tag="daccb", bufs=2,
                                     name=nm("daccb"))
                        nc.vector.tensor_copy(db[:], dacc[j][:])
                        den_ps = psC.tile([1, 512], F32, tag="sc", bufs=4,
                                          name=nm("den"))
                        nc.tensor.matmul(den_ps[:], ones_bf[:], db[:],
                                         start=True, stop=True)
                        rec = sp.tile([1, 512], F32, tag="rec", bufs=2,
                                      name=nm("rec"))
                        nc.vector.reciprocal_approx_fast(rec[:], den_ps[:])
                        bc = sp.tile([128, 512], F32, tag="bc", bufs=2,
                                     name=nm("bc"))
                        nc.gpsimd.partition_broadcast(bc[:], rec[:])
                        ao = sp.tile([128, 512], BF16, tag="ao", bufs=2,
                                     name=nm("ao"))
                        nc.vector.tensor_mul(ao[:], num_ps[j][:], bc[:])
                        if half == 0:
                            nc.sync.dma_start(
                                out=ag2_in[DV * h:DV * (h + 1),
                                           COL_OF_J[j]:COL_OF_J[j] + 512],
                                in_=ao[:])
                        else:
                            nc.sync.dma_start(
                                out=ag2j_in[j][DV * h:DV * (h + 1), :],
                                in_=ao[:])
                            if h == HC - 1 and j == 2:
                                # j2 done on both heads: fire its gather early
                                nc.gpsimd.collective_compute(
                                    "AllGather", mybir.AluOpType.bypass,
                                    replica_groups=rg,
                                    ins=[ag2j_in[2][:]], outs=[ag2j_out[2][:]])

                    pend = []

                    def flush():
                        for (p_sb, j, ii, c0) in pend:
                            nc.tensor.matmul(
                                num_ps[j][:, c0:512],
                                v_sb[ii][:, 128 * h:128 * (h + 1)],
                                p_sb[:], start=(ii == 0),
                                stop=(ii == 4 * j + 3))
                        for (p_sb, j, ii, c0) in pend:
                            if ii == 4 * j + 3:
                                finalize(j)
                        pend.clear()

                    for i in range(imax):
                        cur = [j for j in js if i <= 4 * j + 3]
                        scs = {}
                        for j in cur:
                            q0 = i - 4 * j
                            col0 = 128 * q0 if q0 > 0 else 0
                            sc = psC.tile([128, 512 - col0], F32, tag="sc",
                                          bufs=4, name=nm("sc"))
                            scs[j] = (sc, col0)
                        for j in cur:
                            sc, col0 = scs[j]
                            nc.tensor.matmul(
                                sc[:], kn_sb[h][:, 128 * i:128 * (i + 1)],
                                qn_sb[h][:, 512 * j + col0:512 * (j + 1)],
                                start=True, stop=False)
                        for j in cur:
                            sc, col0 = scs[j]
                            nc.tensor.matmul(
                                sc[:], kpe_sb[:, 128 * i:128 * (i + 1)],
                                qp_sb[h][:, 512 * j + col0:512 * (j + 1)],
                                start=False, stop=True)
                        nxt = []
                        for j in cur:
                            sc, col0 = scs[j]
                            w = 512 - col0
                            p_sb = sp.tile([128, w], BF16, tag="p", bufs=6,
                                           name=nm("p"))
                            nc.scalar.activation(
                                p_sb[:], sc[:],
                                mybir.ActivationFunctionType.Exp, scale=SCALE)
                            if i >= 4 * j:
                                nc.vector.tensor_mul(
                                    p_sb[:, 0:128], p_sb[:, 0:128],
                                    mask_sb[:, 128 * i:128 * (i + 1)])
                            if i == 0:
                                nc.vector.tensor_copy(dacc[j][:], p_sb[:])
                            else:
                                nc.vector.tensor_add(
                                    dacc[j][:, col0:512],
                                    dacc[j][:, col0:512], p_sb[:])
                            nxt.append((p_sb, j, i, col0))
                        flush()
                        pend.extend(nxt)
                    flush()

                if half == 0:
                    nc.gpsimd.collective_compute(
                        "AllGather", mybir.AluOpType.bypass, replica_groups=rg,
                        ins=[ag2_in[:]], outs=[ag2_out[:]])
                else:
                    nc.gpsimd.collective_compute(
                        "AllGather", mybir.AluOpType.bypass, replica_groups=rg,
                        ins=[ag2j_in[3][:]], outs=[ag2j_out[3][:]])

            # ============ B5: o_proj (feature-sharded) =====================
            # half 0: combined [*, 1024] gather; half 1: per-j gathers.
            def oproj(src, jcols):
                njl = len(jcols)
                ops = {}
                for me in range(2):
                    for jl in range(njl):
                        ops[(me, jl)] = psC.tile([128, 512], F32, tag="acc",
                                                 bufs=4, name=nm("op"))
                for k in range(16):
                    rhs = sp.tile([128, 512 * njl], BF16, tag="orhs", bufs=4,
                                  name=nm("orhs"))
                    eng = nc.scalar if k % 2 == 0 else nc.sync
                    eng.dma_start(out=rhs[:],
                                  in_=src[128 * k:128 * (k + 1), :])
                    for me in range(2):
                        for jl in range(njl):
                            nc.tensor.matmul(
                                ops[(me, jl)][:],
                                wo_sb[:, EC * k + 128 * me:EC * k + 128 * (me + 1)],
                                rhs[:, 512 * jl:512 * (jl + 1)],
                                start=(k == 0), stop=(k == 15))
                for jl in range(njl):
                    j = jcols[jl]
                    for me in range(2):
                        yo = sp.tile([128, 512], F32, tag="yo", bufs=2,
                                     name=nm("yo"))
                        nc.vector.tensor_copy(yo[:], ops[(me, jl)][:])
                        nc.sync.dma_start(
                            out=y_out[128 * me:128 * (me + 1),
                                      512 * j:512 * (j + 1)],
                            in_=yo[:])

            oproj(ag2_out, (0, 1))
            oproj(ag2j_out[2], (2,))
            oproj(ag2j_out[3], (3,))

    nc.compile()
    return nc


_PROGRAM = None


def _get_program():
    global _PROGRAM
    if _PROGRAM is None:
        _PROGRAM = build_program()
    return _PROGRAM


def _prep_inputs(positions, hidden_states, w_q_a, q_a_ln_w, w_q_b, w_kv_a,
                 kv_a_ln_w, w_kv_b, w_o):
    pos = np.asarray(positions).astype(np.float32)
    hidden_states = np.asarray(hidden_states, dtype=np.float32)
    w_q_a = np.asarray(w_q_a, dtype=np.float32)
    q_a_ln_w = np.asarray(q_a_ln_w, dtype=np.float32)
    w_q_b = np.asarray(w_q_b, dtype=np.float32)
    w_kv_a = np.asarray(w_kv_a, dtype=np.float32)
    kv_a_ln_w = np.asarray(kv_a_ln_w, dtype=np.float32)
    w_kv_b = np.asarray(w_kv_b, dtype=np.float32)
    w_o = np.asarray(w_o, dtype=np.float32)

    perm = np.concatenate([np.arange(0, DR, 2), np.arange(1, DR, 2)])
    perm2 = np.concatenate([np.arange(1, DR, 2), np.arange(0, DR, 2)])

    inv = 1.0 / (THETA ** (np.arange(0, DR, 2, dtype=np.float32) / DR))
    f = pos[:, None] * inv[None, :]                      # [T, 32]
    cos_t = np.ascontiguousarray(np.cos(f).astype(np.float32).T)
    sin_t = np.ascontiguousarray(np.sin(f).astype(np.float32).T)
    csf_t = np.ascontiguousarray(np.tile(cos_t, (4, 1))).astype(NPBF16)
    snf_t = np.ascontiguousarray(
        np.concatenate([-sin_t, sin_t, -sin_t, sin_t], axis=0)).astype(NPBF16)

    # diag-block keep masks: for s-tile i, t-super i//4
    masks = np.zeros((16, 128, 128), dtype=NPBF16)
    for i in range(16):
        s_idx = pos[128 * i:128 * (i + 1)]
        masks[i] = (s_idx[None, :] >= s_idx[:, None]).astype(NPBF16)

    # pack (J,k) tiles contiguous: row ((16J+k)*128+p) = w_q_a.T[128k+p, 512J+e]
    w_q_aT = np.ascontiguousarray(
        w_q_a.T.reshape(16, 128, 3, 512).transpose(2, 0, 1, 3)
        .reshape(48 * 128, 512)).astype(NPBF16)
    w_kv_aT = np.ascontiguousarray(w_kv_a.T)                     # [2048, 576]
    w_kv_aT[:, KVL:] = w_kv_aT[:, KVL:][:, perm]
    w_kv_aT = w_kv_aT.astype(NPBF16)

    hid_T_full = np.ascontiguousarray(hidden_states.T).astype(NPBF16)

    in_maps = []
    for c in range(NC):
        h0, h1 = 2 * c, 2 * c + 1
        Wh0 = w_q_b[h0 * (DN + DR):(h0 + 1) * (DN + DR), :]      # [192, 1536]
        Wh1 = w_q_b[h1 * (DN + DR):(h1 + 1) * (DN + DR), :]
        blocks = [Wh0[:DN], Wh1[:DN], Wh0[DN:][perm], Wh1[DN:][perm],
                  Wh0[DN:][perm2], Wh1[DN:][perm2]]
        w_qbT = (np.concatenate(blocks, axis=0).T
                 * q_a_ln_w[:, None]).astype(NPBF16)             # [1536, 384]
        w_kbT = (np.concatenate(
            [w_kv_b[h * (DN + DV):h * (DN + DV) + DN] for h in (h0, h1)],
            axis=0).T * kv_a_ln_w[:, None]).astype(NPBF16)       # [512, 256]
        w_vbT = (np.concatenate(
            [w_kv_b[h * (DN + DV) + DN:(h + 1) * (DN + DV)] for h in (h0, h1)],
            axis=0).T * kv_a_ln_w[:, None]).astype(NPBF16)       # [512, 256]
        w_oTc = np.ascontiguousarray(w_o.T[:, EC * c:EC * (c + 1)]).astype(NPBF16)
        # hid chunk, rows permuted p-major: dram row (16 p + k) = orig 128 k + p
        X = hid_T_full[:, TC * c:TC * (c + 1)]
        hid_pm = np.ascontiguousarray(
            X.reshape(16, 128, TC).transpose(1, 0, 2).reshape(HID, TC))
        in_maps.append({
            "hid_T": hid_pm,
            "w_q_aT": w_q_aT,
            "w_kv_aT": w_kv_aT,
            "w_qbT": np.ascontiguousarray(w_qbT),
            "w_kbT": np.ascontiguousarray(w_kbT),
            "w_vbT": np.ascontiguousarray(w_vbT),
            "w_oT": w_oTc,
            "csf_t": csf_t,
            "snf_t": snf_t,
            "cos_c": np.ascontiguousarray(cos_t[:, TC * c:TC * (c + 1)]),
            "sin_c": np.ascontiguousarray(sin_t[:, TC * c:TC * (c + 1)]),
            "masks": masks,
        })
    return in_maps


RUN_KWARGS = {}
LAST_RESULT = None


def kernel(**inputs):
    global LAST_RESULT
    nc = _get_program()
    in_maps = _prep_inputs(**inputs)
    res = run_bass_kernel_spmd(nc, in_maps, list(range(NC)), **RUN_KWARGS)
    LAST_RESULT = res
    yT = np.concatenate([res.results[c]["y_out"] for c in range(NC)], axis=0)
    return np.ascontiguousarray(yT.T)


# revision 31
# speedup vs baseline: 1.2166x; 1.0030x over previous
"""DeepseekV2 MLA attention fusion (prefill, T=2048) on 8 Trainium2 cores.

Sharding:
  - Phase A (q_a / kv_a projections + rmsnorm + k_pe rope): sharded over tokens
    (256 tokens/core), feature-major outputs.  kv/k_pe AllGathered early
    (bf16), q_c AllGathered in two row-halves as fp8e4 (the rmsnormed q_c is
    O(1), e4m3 transport keeps rel err ~1e-2) so the dominant collective is
    half the bytes.
  - Phase B tensor-parallel over heads (2 heads/core).  kv_b expansion (B2)
    overlaps the q AllGather; q_b projection (B1) consumes the fp8 q_c
    directly as the matmul moving operand, with a packed rope stationary
    (3 matmuls/k-tile for 384 output cols) and natural k-split accumulation
    so its first half starts as soon as agq half-a lands.
  - Attention runs per 512-token j-supertile in DESCENDING order (j3..j0);
    each j's normalized output is AllGathered (bf16) immediately and its
    o_proj (feature-sharded, 256 cols/core) queues behind the remaining
    attention work, so the final gather overlaps queued o_proj matmuls.
  - softmax denominator: fp32 vector accumulation of the exp tiles + a single
    ones-matmul per (j,h) instead of a PE matmul per s-tile.

All matmuls bf16 (q_b: bf16 stationary x fp8 moving) with fp32 PSUM
accumulation; softmax in fp32.
"""
import sys

sys.path.insert(0, "/opt/trn_rl_repo")

from contextlib import ExitStack

import ml_dtypes
import numpy as np

from concourse import bacc, bass, mybir, tile
from concourse.bass_utils import run_bass_kernel_spmd

BF16 = mybir.dt.bfloat16
F32 = mybir.dt.float32
FP8 = mybir.dt.float8e4
NPBF16 = ml_dtypes.bfloat16
NPFP8 = ml_dtypes.float8_e4m3

T = 2048
HID = 2048
H = 16
DN = 128
DR = 64
DV = 128
QL = 1536
KVL = 512
KVR = KVL + DR
THETA = 10000.0
EPS = 1e-6
SCALE = (DN + DR) ** -0.5

NC = 8            # cores
TC = T // NC      # tokens per core = 256
HC = H // NC      # heads per core = 2
EC = HID // NC    # o_proj output cols per core = 256


def build_program():
    nc = bacc.Bacc("TRN2", target_bir_lowering=False, debug=False, num_devices=NC)

    _ctr = [0]

    def nm(tag):
        _ctr[0] += 1
        return f"t{_ctr[0]}_{tag}"

    def din(name, shape, dt=BF16):
        return nc.dram_tensor(name, shape, dt, kind="ExternalInput").ap()

    hid_T = din("hid_T", [HID, TC])                 # hidden.T chunk, p-major rows
    w_q_aT = din("w_q_aT", [48 * 128, 512])         # packed (J,k) tiles
    w_kv_aT = din("w_kv_aT", [HID, KVR])            # rope cols de-interleaved
    w_qbT = din("w_qbT", [QL, 4 * 128])             # [h0n, h1n, rope, rope_sw]
    w_kbT = din("w_kbT", [KVL, 2 * DN])             # [h0 k, h1 k]
    w_vbT = din("w_vbT", [KVL, 2 * DV])             # [h0 v, h1 v]
    w_oT = din("w_oT", [H * DV, EC])
    csf_t = din("csf_t", [128, T])                  # cos x4 groups (bf16)
    snf_t = din("snf_t", [128, T])                  # [-sin,sin,-sin,sin]
    cos_c = din("cos_c", [DR // 2, TC], F32)        # core's chunk (k rope)
    sin_c = din("sin_c", [DR // 2, TC], F32)
    masks = din("masks", [16, 128, 128])            # diag triangles

    y_out = nc.dram_tensor("y_out", [EC, T], F32, kind="ExternalOutput").ap()

    agkv_in = nc.dram_tensor("agkv_in", [KVR, TC], BF16).ap()
    agkv_out = nc.dram_tensor("agkv_out", [NC * KVR, TC], BF16,
                              addr_space="Shared").ap()
    agq_in = nc.dram_tensor("agq_in", [QL, TC], FP8).ap()
    agq_out = nc.dram_tensor("agq_out", [NC * QL, TC], FP8,
                             addr_space="Shared").ap()
    ag2_in = nc.dram_tensor("ag2a_in", [HC * DV, 1024], BF16).ap()
    ag2_out = nc.dram_tensor("ag2a_out", [H * DV, 1024], BF16,
                             addr_space="Shared").ap()
    ag2j_in = {j: nc.dram_tensor(f"ag2{j}_in", [HC * DV, 512], BF16).ap()
               for j in (2, 3)}
    ag2j_out = {j: nc.dram_tensor(f"ag2{j}_out", [H * DV, 512], BF16,
                                  addr_space="Shared").ap() for j in (2, 3)}

    rg = [list(range(NC))]

    with tile.TileContext(nc) as tc, ExitStack() as ctx:
        const = ctx.enter_context(tc.tile_pool(name="const", bufs=1))
        sp = ctx.enter_context(tc.tile_pool(name="sp", bufs=2))

        ones_bf = const.tile([128, 1], BF16, tag="ones_bf", name=nm("ones_bf"))
        nc.vector.memset(ones_bf[:], 1.0)

        # ============ Phase A (token-sharded, feature-major outputs) =======
        ones_col = const.tile([128, 1], F32, tag="ones_col", name=nm("ones_col"))
        nc.vector.memset(ones_col[:], 1.0)
        eps1_sb = const.tile([1, 1], F32, tag="eps1", name=nm("eps1"))
        nc.vector.memset(eps1_sb[:], EPS)

        with tc.tile_pool(name="pA", bufs=1) as pA, \
             tc.tile_pool(name="psA", bufs=1, space="PSUM") as psA:
            # rows pre-permuted host-side: dram row (16 p + k) = contraction
            # row (128 k + p), so each chunk is contiguous per partition.
            hid4 = hid_T.rearrange("(p g k) t -> p g (k t)", p=128, g=4)
            hid_sb = []
            for g in range(4):
                t = pA.tile([128, 4 * TC], BF16, tag=f"hid{g}",
                            name=nm(f"hid{g}"))
                nc.sync.dma_start(out=t[:], in_=hid4[:, g, :])
                hid_sb.append(t)

            def hid_k(k):
                g, kk = divmod(k, 4)
                return hid_sb[g][:, TC * kk:TC * (kk + 1)]

            def norm_scale(sumsq, n_feat):
                """rsqrt(mean+eps) broadcast to [128, TC] SBUF tile."""
                s = sp.tile([1, TC], F32, tag="nrm", bufs=2, name=nm("nrm"))
                nc.scalar.activation(s[:], sumsq[:],
                                     mybir.ActivationFunctionType.Sqrt,
                                     bias=eps1_sb[:], scale=1.0 / n_feat)
                r = sp.tile([1, TC], F32, tag="nrm_r", bufs=2, name=nm("nrm_r"))
                nc.vector.reciprocal_approx_fast(r[:], s[:])
                bc = sp.tile([128, TC], F32, tag="nrm_bc", bufs=2,
                             name=nm("nrm_bc"))
                nc.gpsimd.partition_broadcast(bc[:], r[:])
                return bc

            # ---- latent: kv_c 4 blocks + k_pe, fused per-k loop ----
            kv_ps = [psA.tile([128, TC], F32, tag="a_ps", bufs=4,
                              name=nm("a_ps")) for _ in range(4)]
            pe_ps = psA.tile([64, TC], F32, tag="pe_ps", bufs=1, name=nm("pe_ps"))
            for k in range(16):
                wt = sp.tile([128, KVR], BF16, tag="wA", bufs=4, name=nm("wA"))
                nc.scalar.dma_start(out=wt[:],
                                    in_=w_kv_aT[128 * k:128 * (k + 1), :])
                for jj in range(4):
                    nc.tensor.matmul(kv_ps[jj][:], wt[:, 128 * jj:128 * (jj + 1)],
                                     hid_k(k), start=(k == 0), stop=(k == 15))
                nc.tensor.matmul(pe_ps[:], wt[:, KVL:KVR], hid_k(k),
                                 start=(k == 0), stop=(k == 15))
            kv_sq = psA.tile([1, TC], F32, tag="sumsq", bufs=2, name=nm("sumsq"))
            kvc_sb = []
            for jj in range(4):
                sq = sp.tile([128, TC], F32, tag="sq", bufs=2, name=nm("sq"))
                nc.scalar.square(sq[:], kv_ps[jj][:])
                o = pA.tile([128, TC], F32, tag=f"kvc{jj}", name=nm("kvc"))
                nc.vector.tensor_copy(o[:], kv_ps[jj][:])
                kvc_sb.append(o)
                nc.tensor.matmul(kv_sq[:], ones_col[:], sq[:],
                                 start=(jj == 0), stop=(jj == 3))
            kbc = norm_scale(kv_sq, KVL)
            for jj in range(4):
                o = sp.tile([128, TC], BF16, tag="agst", bufs=3, name=nm("agst"))
                nc.vector.tensor_mul(o[:], kvc_sb[jj][:], kbc[:])
                nc.sync.dma_start(out=agkv_in[128 * jj:128 * (jj + 1), :],
                                  in_=o[:])
            # k_pe rope (partition pairs 0:32 / 32:64, de-interleaved)
            cosc = pA.tile([32, TC], F32, tag="cosc", name=nm("cosc"))
            sinc = pA.tile([32, TC], F32, tag="sinc", name=nm("sinc"))
            nc.sync.dma_start(out=cosc[:], in_=cos_c[:])
            nc.sync.dma_start(out=sinc[:], in_=sin_c[:])
            kpe_ro = sp.tile([64, TC], BF16, tag="kpero", bufs=1, name=nm("kpero"))
            t1 = sp.tile([32, TC], F32, tag="ro1", bufs=2, name=nm("ro1"))
            t2 = sp.tile([32, TC], F32, tag="ro2", bufs=2, name=nm("ro2"))
            nc.vector.tensor_mul(t1[:], pe_ps[0:32, :], cosc[:])
            nc.vector.tensor_mul(t2[:], pe_ps[32:64, :], sinc[:])
            nc.vector.tensor_sub(kpe_ro[0:32, :], t1[:], t2[:])
            t3 = sp.tile([32, TC], F32, tag="ro1", bufs=2, name=nm("ro1"))
            t4 = sp.tile([32, TC], F32, tag="ro2", bufs=2, name=nm("ro2"))
            nc.vector.tensor_mul(t3[:], pe_ps[32:64, :], cosc[:])
            nc.vector.tensor_mul(t4[:], pe_ps[0:32, :], sinc[:])
            nc.vector.tensor_add(kpe_ro[32:64, :], t3[:], t4[:])
            nc.sync.dma_start(out=agkv_in[KVL:KVR, :], in_=kpe_ro[:])

            nc.gpsimd.collective_compute(
                "AllGather", mybir.AluOpType.bypass, replica_groups=rg,
                ins=[agkv_in[:]], outs=[agkv_out[:]])

            # ---- q_c: 12 blocks in 3 J-supertiles; fp8 transport ----
            q_sq = psA.tile([1, TC], F32, tag="sumsq", bufs=2, name=nm("sumsq"))
            qc_sb = []
            for J in range(3):
                pss = [psA.tile([128, TC], F32, tag="a_ps", bufs=4,
                                name=nm("a_ps")) for _ in range(4)]
                for k in range(16):
                    wt = sp.tile([128, 512], BF16, tag="wAq", bufs=4,
                                 name=nm("wAq"))
                    r0 = (16 * J + k) * 128
                    eng = nc.sync if k % 2 == 0 else nc.scalar
                    eng.dma_start(out=wt[:], in_=w_q_aT[r0:r0 + 128, :])
                    for jj in range(4):
                        nc.tensor.matmul(pss[jj][:],
                                         wt[:, 128 * jj:128 * (jj + 1)],
                                         hid_k(k), start=(k == 0), stop=(k == 15))
                for jj in range(4):
                    j = 4 * J + jj
                    sq = sp.tile([128, TC], F32, tag="sq", bufs=2, name=nm("sq"))
                    nc.scalar.square(sq[:], pss[jj][:])
                    o = pA.tile([128, TC], F32, tag=f"qc{j}", name=nm("qc"))
                    nc.vector.tensor_copy(o[:], pss[jj][:])
                    qc_sb.append(o)
                    nc.tensor.matmul(q_sq[:], ones_col[:], sq[:],
                                     start=(j == 0), stop=(j == 11))
            qbc = norm_scale(q_sq, QL)
            for j in range(12):
                o = sp.tile([128, TC], FP8, tag="agstq", bufs=3,
                            name=nm("agstq"))
                nc.vector.tensor_mul(o[:], qc_sb[j][:], qbc[:])
                nc.sync.dma_start(
                    out=agq_in[128 * j:128 * (j + 1), :], in_=o[:])
            nc.gpsimd.collective_compute(
                "AllGather", mybir.AluOpType.bypass, replica_groups=rg,
                ins=[agq_in[:]], outs=[agq_out[:]])

        # ============ Phase B resident tensors =============================
        pB = ctx.enter_context(tc.tile_pool(name="pB", bufs=1))
        csf_sb = pB.tile([128, T], BF16, tag="csf", name=nm("csf"))
        snf_sb = pB.tile([128, T], BF16, tag="snf", name=nm("snf"))
        nc.sync.dma_start(out=csf_sb[:], in_=csf_t[:])
        nc.sync.dma_start(out=snf_sb[:], in_=snf_t[:])

        wkb_sb = pB.tile([128, 4 * 256], BF16, tag="wkb", name=nm("wkb"))
        nc.sync.dma_start(out=wkb_sb[:],
                          in_=w_kbT.rearrange("(k p) e -> p k e", p=128))
        wvb_sb = pB.tile([128, 4 * 256], BF16, tag="wvb", name=nm("wvb"))
        nc.sync.dma_start(out=wvb_sb[:],
                          in_=w_vbT.rearrange("(k p) e -> p k e", p=128))
        wqb_sb = pB.tile([128, 12 * 512], BF16, tag="wqb", name=nm("wqb"))
        nc.sync.dma_start(out=wqb_sb[:],
                          in_=w_qbT.rearrange("(k p) e -> p k e", p=128))
        mask_sb = pB.tile([128, 16 * 128], BF16, tag="mask", name=nm("mask"))
        nc.sync.dma_start(out=mask_sb[:],
                          in_=masks.rearrange("i p f -> p i f"))
        wo_sb = pB.tile([128, 16 * EC], BF16, tag="wo", name=nm("wo"))
        nc.sync.dma_start(out=wo_sb[:],
                          in_=w_oT.rearrange("(k p) e -> p k e", p=128))

        def wkb(k):
            return wkb_sb[:, 256 * k:256 * (k + 1)]

        def wvb(k):
            return wvb_sb[:, 256 * k:256 * (k + 1)]

        def wqb(k):
            return wqb_sb[:, 512 * k:512 * (k + 1)]

        # batched loads of the gathered activations (feature-major)
        agkv3 = agkv_out.rearrange("(c r) t -> r c t", c=NC)   # [576, 8, 256]
        kv_sb = []
        for i in range(4):
            t = pB.tile([128, T], BF16, tag=f"kv{i}", name=nm(f"kv{i}"))
            nc.scalar.dma_start(out=t[:], in_=agkv3[128 * i:128 * (i + 1)])
            kv_sb.append(t)
        kpe_sb = pB.tile([64, T], BF16, tag="kpe", name=nm("kpe"))
        nc.scalar.dma_start(out=kpe_sb[:], in_=agkv3[KVL:KVR])
        agq3 = agq_out.rearrange("(c r) t -> r c t", c=NC)   # [1536, 8, 256]
        qA = []
        for k in range(12):
            t = pB.tile([128, T], FP8, tag=f"qA{k}", name=nm(f"qA{k}"))
            nc.scalar.dma_start(out=t[:], in_=agq3[128 * k:128 * (k + 1)])
            qA.append(t)

        qn_sb = [pB.tile([128, T], BF16, tag=f"qn{h}", name=nm(f"qn{h}"))
                 for h in range(HC)]
        qp_sb = [pB.tile([64, T], BF16, tag=f"qp{h}", name=nm(f"qp{h}"))
                 for h in range(HC)]
        kn_sb = [pB.tile([128, T], BF16, tag=f"kn{h}", name=nm(f"kn{h}"))
                 for h in range(HC)]
        v_sb = [pB.tile([128, 2 * DV], BF16, tag=f"v{i}", name=nm(f"v{i}"))
                for i in range(16)]

        with tc.tile_pool(name="psB2", bufs=1, space="PSUM") as psB:
            # ============ B2: kv_b expansion (overlaps the q AllGather) ====
            for h in range(HC):
                for jp in range(2):           # j-pairs, lhsT reused across js
                    kn_ps = [psB.tile([128, 512], F32, tag="b2", bufs=2,
                                      name=nm("kn_ps")) for _ in range(2)]
                    for i in range(4):
                        for jj in range(2):
                            j = 2 * jp + jj
                            nc.tensor.matmul(
                                kn_ps[jj][:], wkb(i)[:, 128 * h:128 * (h + 1)],
                                kv_sb[i][:, 512 * j:512 * (j + 1)],
                                start=(i == 0), stop=(i == 3))
                    for jj in range(2):
                        j = 2 * jp + jj
                        nc.vector.tensor_copy(
                            kn_sb[h][:, 512 * j:512 * (j + 1)], kn_ps[jj][:])
            for s16 in range(16):
                v_ps = psB.tile([128, 2 * DV], F32, tag="b2", bufs=2,
                                name=nm("v_ps"))
                for i in range(4):
                    nc.tensor.matmul(v_ps[:],
                                     kv_sb[i][:, 128 * s16:128 * (s16 + 1)],
                                     wvb(i)[:], start=(i == 0), stop=(i == 3))
                nc.vector.tensor_copy(v_sb[s16][:], v_ps[:])

        # ============ B1: q_b projection + q rope ======================
        with tc.tile_pool(name="psB1", bufs=1, space="PSUM") as psB1:
            for js in ((0, 1), (2, 3)):
                qn_ps = {}
                qp_ps = {}
                qps_ps = {}
                for j in js:
                    qn_ps[(j, 0)] = psB1.tile([128, 512], F32, tag="b1qn",
                                              bufs=4, name=nm("qn_ps"))
                    qn_ps[(j, 1)] = psB1.tile([128, 512], F32, tag="b1qn",
                                              bufs=4, name=nm("qn_ps"))
                    qp_ps[j] = psB1.tile([128, 512], F32, tag="b1qp",
                                         bufs=2, name=nm("qp_ps"))
                    qps_ps[j] = psB1.tile([128, 512], F32, tag="b1qs",
                                          bufs=2, name=nm("qps_ps"))
                for k in range(12):
                    for m in range(4):        # stationary reused across js
                        for j in js:
                            dst = (qn_ps[(j, 0)], qn_ps[(j, 1)],
                                   qp_ps[j], qps_ps[j])[m]
                            nc.tensor.matmul(
                                dst[:], wqb(k)[:, 128 * m:128 * (m + 1)],
                                qA[k][:, 512 * j:512 * (j + 1)],
                                start=(k == 0), stop=(k == 11))
                for j in js:
                    for h in range(HC):
                        nc.vector.tensor_copy(
                            qn_sb[h][:, 512 * j:512 * (j + 1)],
                            qn_ps[(j, h)][:])
                    # rope: qp2 = qp * cos4 + qp_swapped * (+-sin)
                    t1 = sp.tile([128, 512], F32, tag="ro1", bufs=2,
                                 name=nm("ro1"))
                    t2 = sp.tile([128, 512], F32, tag="ro2", bufs=2,
                                 name=nm("ro2"))
                    nc.vector.tensor_mul(t1[:], qp_ps[j][:],
                                         csf_sb[:, 512 * j:512 * (j + 1)])
                    nc.vector.tensor_mul(t2[:], qps_ps[j][:],
                                         snf_sb[:, 512 * j:512 * (j + 1)])
                    for h in range(HC):
                        nc.vector.tensor_add(
                            qp_sb[h][:, 512 * j:512 * (j + 1)],
                            t1[64 * h:64 * (h + 1), :],
                            t2[64 * h:64 * (h + 1), :])

        # ============ B3: attention, two phases of two interleaved js ======
        PHASES = [(0, (0, 1), 8), (1, (2, 3), 16)]
        COL_OF_J = {0: 0, 1: 512, 2: 0, 3: 512}
        with tc.tile_pool(name="psC", bufs=1, space="PSUM") as psC:
            for half, js, imax in PHASES:
                for h in range(HC):
                    num_ps = {j: psC.tile([128, 512], F32, tag="acc", bufs=4,
                                          name=nm("num")) for j in js}
                    dacc = {j: sp.tile([128, 512], F32, tag="dacc", bufs=4,
                                       name=nm("dacc")) for j in js}

                    def finalize(j):
                        db = sp.tile([128, 512], BF16, tag="daccb", bufs=2,
                                     name=nm("daccb"))
                        nc.vector.tensor_copy(db[:], dacc[j][:])
                        den_ps = psC.tile([1, 512], F32, tag="sc", bufs=4,
                                          name=nm("den"))
                        nc.tensor.matmul(den_ps[:], ones_bf[:], db[:],
                                         start=True, stop=True)
                        rec = sp.tile([1, 512], F32, tag="rec", bufs=2,
                                      name=nm("rec"))
                        nc.vector.reciprocal_approx_fast(rec[:], den_ps[:])
                        bc = sp.tile([128, 512], F32, tag="bc", bufs=2,
                                     name=nm("bc"))
                        nc.gpsimd.partition_broadcast(bc[:], rec[:])
                        ao = sp.tile([128, 512], BF16, tag="ao", bufs=2,
                                     name=nm("ao"))
                        nc.vector.tensor_mul(ao[:], num_ps[j][:], bc[:])
                        if half == 0:
                            nc.sync.dma_start(
                                out=ag2_in[DV * h:DV * (h + 1),
                                           COL_OF_J[j]:COL_OF_J[j] + 512],
                                in_=ao[:])
                        else:
                            nc.sync.dma_start(
                                out=ag2j_in[j][DV * h:DV * (h + 1), :],
                                in_=ao[:])
                            if h == HC - 1 and j == 2:
                                # j2 done on both heads: fire its gather early
                                nc.gpsimd.collective_compute(
                                    "AllGather", mybir.AluOpType.bypass,
                                    replica_groups=rg,
                                    ins=[ag2j_in[2][:]], outs=[ag2j_out[2][:]])

                    pend = []

                    def flush():
                        for (p_sb, j, ii, c0) in pend:
                            nc.tensor.matmul(
                                num_ps[j][:, c0:512],
                                v_sb[ii][:, 128 * h:128 * (h + 1)],
                                p_sb[:], start=(ii == 0),
                                stop=(ii == 4 * j + 3))
                        for (p_sb, j, ii, c0) in pend:
                            if ii == 4 * j + 3:
                                finalize(j)
                        pend.clear()

                    for i in range(imax):
                        cur = [j for j in js if i <= 4 * j + 3]
                        scs = {}
                        for j in cur:
                            q0 = i - 4 * j
                            col0 = 128 * q0 if q0 > 0 else 0
                            sc = psC.tile([128, 512 - col0], F32, tag="sc",
                                          bufs=4, name=nm("sc"))
                            scs[j] = (sc, col0)
                        for j in cur:
                            sc, col0 = scs[j]
                            nc.tensor.matmul(
                                sc[:], kn_sb[h][:, 128 * i:128 * (i + 1)],
                                qn_sb[h][:, 512 * j + col0:512 * (j + 1)],
                                start=True, stop=False)
                        for j in cur:
                            sc, col0 = scs[j]
                            nc.tensor.matmul(
                                sc[:], kpe_sb[:, 128 * i:128 * (i + 1)],
                                qp_sb[h][:, 512 * j + col0:512 * (j + 1)],
                                start=False, stop=True)
                        nxt = []
                        for j in cur:
                            sc, col0 = scs[j]
                            w = 512 - col0
                            p_sb = sp.tile([128, w], BF16, tag="p", bufs=6,
                                           name=nm("p"))
                            nc.scalar.activation(
                                p_sb[:], sc[:],
                                mybir.ActivationFunctionType.Exp, scale=SCALE)
                            if i >= 4 * j:
                                nc.vector.tensor_mul(
                                    p_sb[:, 0:128], p_sb[:, 0:128],
                                    mask_sb[:, 128 * i:128 * (i + 1)])
                            if i == 0:
                                nc.vector.tensor_copy(dacc[j][:], p_sb[:])
                            else:
                                nc.vector.tensor_add(
                                    dacc[j][:, col0:512],
                                    dacc[j][:, col0:512], p_sb[:])
                            nxt.append((p_sb, j, i, col0))
                        flush()
                        pend.extend(nxt)
                    flush()

                if half == 0:
                    nc.gpsimd.collective_compute(
                        "AllGather", mybir.AluOpType.bypass, replica_groups=rg,
                        ins=[ag2_in[:]], outs=[ag2_out[:]])
                else:
                    nc.gpsimd.collective_compute(
                        "AllGather", mybir.AluOpType.bypass, replica_groups=rg,
                        ins=[ag2j_in[3][:]], outs=[ag2j_out[3][:]])

            # ============ B5: o_proj (feature-sharded) =====================
            # half 0: combined [*, 1024] gather; half 1: per-j gathers.
            def oproj(src, jcols):
                njl = len(jcols)
                ops = {}
                for me in range(2):
                    for jl in range(njl):
                        ops[(me, jl)] = psC.tile([128, 512], F32, tag="acc",
                                                 bufs=4, name=nm("op"))
                for k in range(16):
                    rhs = sp.tile([128, 512 * njl], BF16, tag="orhs", bufs=4,
                                  name=nm("orhs"))
                    eng = nc.scalar if k % 2 == 0 else nc.sync
                    eng.dma_start(out=rhs[:],
                                  in_=src[128 * k:128 * (k + 1), :])
                    for me in range(2):
                        for jl in range(njl):
                            nc.tensor.matmul(
                                ops[(me, jl)][:],
                                wo_sb[:, EC * k + 128 * me:EC * k + 128 * (me + 1)],
                                rhs[:, 512 * jl:512 * (jl + 1)],
                                start=(k == 0), stop=(k == 15))
                for jl in range(njl):
                    j = jcols[jl]
                    for me in range(2):
                        yo = sp.tile([128, 512], F32, tag="yo", bufs=2,
                                     name=nm("yo"))
                        nc.vector.tensor_copy(yo[:], ops[(me, jl)][:])
                        nc.sync.dma_start(
                            out=y_out[128 * me:128 * (me + 1),
                                      512 * j:512 * (j + 1)],
                            in_=yo[:])

            oproj(ag2_out, (0, 1))
            oproj(ag2j_out[2], (2,))
            oproj(ag2j_out[3], (3,))

    nc.compile()
    return nc


_PROGRAM = None


def _get_program():
    global _PROGRAM
    if _PROGRAM is None:
        _PROGRAM = build_program()
    return _PROGRAM


def _prep_inputs(positions, hidden_states, w_q_a, q_a_ln_w, w_q_b, w_kv_a,
                 kv_a_ln_w, w_kv_b, w_o):
    pos = np.asarray(positions).astype(np.float32)
    hidden_states = np.asarray(hidden_states, dtype=np.float32)
    w_q_a = np.asarray(w_q_a, dtype=np.float32)
    q_a_ln_w = np.asarray(q_a_ln_w, dtype=np.float32)
    w_q_b = np.asarray(w_q_b, dtype=np.float32)
    w_kv_a = np.asarray(w_kv_a, dtype=np.float32)
    kv_a_ln_w = np.asarray(kv_a_ln_w, dtype=np.float32)
    w_kv_b = np.asarray(w_kv_b, dtype=np.float32)
    w_o = np.asarray(w_o, dtype=np.float32)

    perm = np.concatenate([np.arange(0, DR, 2), np.arange(1, DR, 2)])
    perm2 = np.concatenate([np.arange(1, DR, 2), np.arange(0, DR, 2)])

    inv = 1.0 / (THETA ** (np.arange(0, DR, 2, dtype=np.float32) / DR))
    f = pos[:, None] * inv[None, :]                      # [T, 32]
    cos_t = np.ascontiguousarray(np.cos(f).astype(np.float32).T)
    sin_t = np.ascontiguousarray(np.sin(f).astype(np.float32).T)
    csf_t = np.ascontiguousarray(np.tile(cos_t, (4, 1))).astype(NPBF16)
    snf_t = np.ascontiguousarray(
        np.concatenate([-sin_t, sin_t, -sin_t, sin_t], axis=0)).astype(NPBF16)

    # diag-block keep masks: for s-tile i, t-super i//4
    masks = np.zeros((16, 128, 128), dtype=NPBF16)
    for i in range(16):
        s_idx = pos[128 * i:128 * (i + 1)]
        masks[i] = (s_idx[None, :] >= s_idx[:, None]).astype(NPBF16)

    # pack (J,k) tiles contiguous: row ((16J+k)*128+p) = w_q_a.T[128k+p, 512J+e]
    w_q_aT = np.ascontiguousarray(
        w_q_a.T.reshape(16, 128, 3, 512).transpose(2, 0, 1, 3)
        .reshape(48 * 128, 512)).astype(NPBF16)
    w_kv_aT = np.ascontiguousarray(w_kv_a.T)                     # [2048, 576]
    w_kv_aT[:, KVL:] = w_kv_aT[:, KVL:][:, perm]
    w_kv_aT = w_kv_aT.astype(NPBF16)

    hid_T_full = np.ascontiguousarray(hidden_states.T).astype(NPBF16)

    in_maps = []
    for c in range(NC):
        h0, h1 = 2 * c, 2 * c + 1
        Wh0 = w_q_b[h0 * (DN + DR):(h0 + 1) * (DN + DR), :]      # [192, 1536]
        Wh1 = w_q_b[h1 * (DN + DR):(h1 + 1) * (DN + DR), :]
        blocks = [Wh0[:DN], Wh1[:DN], Wh0[DN:][perm], Wh1[DN:][perm],
                  Wh0[DN:][perm2], Wh1[DN:][perm2]]
        w_qbT = (np.concatenate(blocks, axis=0).T
                 * q_a_ln_w[:, None]).astype(NPBF16)             # [1536, 384]
        w_kbT = (np.concatenate(
            [w_kv_b[h * (DN + DV):h * (DN + DV) + DN] for h in (h0, h1)],
            axis=0).T * kv_a_ln_w[:, None]).astype(NPBF16)       # [512, 256]
        w_vbT = (np.concatenate(
            [w_kv_b[h * (DN + DV) + DN:(h + 1) * (DN + DV)] for h in (h0, h1)],
            axis=0).T * kv_a_ln_w[:, None]).astype(NPBF16)       # [512, 256]
        w_oTc = np.ascontiguousarray(w_o.T[:, EC * c:EC * (c + 1)]).astype(NPBF16)
        # hid chunk, rows permuted p-major: dram row (16 p + k) = orig 128 k + p
        X = hid_T_full[:, TC * c:TC * (c + 1)]
        hid_pm = np.ascontiguousarray(
            X.reshape(16, 128, TC).transpose(1, 0, 2).reshape(HID, TC))
        in_maps.append({
            "hid_T": hid_pm,
            "w_q_aT": w_q_aT,
            "w_kv_aT": w_kv_aT,
            "w_qbT": np.ascontiguousarray(w_qbT),
            "w_kbT": np.ascontiguousarray(w_kbT),
            "w_vbT": np.ascontiguousarray(w_vbT),
            "w_oT": w_oTc,
            "csf_t": csf_t,
            "snf_t": snf_t,
            "cos_c": np.ascontiguousarray(cos_t[:, TC * c:TC * (c + 1)]),
            "sin_c": np.ascontiguousarray(sin_t[:, TC * c:TC * (c + 1)]),
            "masks": masks,
        })
    return in_maps


RUN_KWARGS = {}
LAST_RESULT = None


def kernel(**inputs):
    global LAST_RESULT
    nc = _get_program()
    in_maps = _prep_inputs(**inputs)
    res = run_bass_kernel_spmd(nc, in_maps, list(range(NC)), **RUN_KWARGS)
    LAST_RESULT = res
    yT = np.concatenate([res.results[c]["y_out"] for c in range(NC)], axis=0)
    return np.ascontiguousarray(yT.T)
